# revision 9
# baseline (speedup 1.0000x reference)
"""Trainium2 Bass kernel for nn_GroundPropagation.

Structure (8 NeuronCores, batch-parallel, one batch element per core):

Phase 1 (device): per-channel reductions of s = sigmoid(x):
  - row sums  (C, H)  : sum over W of s           -> host computes disp/depth dots
  - sum of s^2 (C,)   : for the channel norms
Host: combines per-core partials in f64, ranks channels by cosine
  similarity against the disparity/depth ramps, picks top-16 + top-16.

Phase 2 (device): for the 32 selected channels, the 32-iteration masked
  "pull value from the row below" propagation collapses algebraically to
  a single bottom-up first-order recurrence per column:
      state = m[t] * state + (1 - m[t]) * sel[t]
  which is exactly one `tensor_tensor_scan` instruction per core
  (columns are packed per-partition; the mask is forced to 0 at each
  column's bottom row so the state resets at column boundaries).
  This is exact whenever no column has >= 33 consecutive masked rows
  (checked on host; P ~ 2^-33 per site otherwise).
  Then the clip-normalized blend weight and the final mix are computed
  and written back; host scatters the 32 channels into the full tensor.
"""

import sys

sys.path.insert(0, "/opt/trn_rl_repo")

import numpy as np

B, C, H, W = 8, 128, 96, 320
HW = H * W                  # 30720
NSEL = 16
NS = 2 * NSEL               # 32 selected channels
CLIP = 0.3
EPS = 1e-6
N_CORES = 8

NCH1, CH1 = 8, HW // 8      # phase-1 chunks (128, 3840)
WQ = 4                      # w-quarters; partition p = wq*32 + ch
WPQ = W // WQ               # 80 columns per quarter
S2 = WPQ * H                # 7680 free elems per partition in phase 2
NCH2 = 4
CH2 = S2 // NCH2            # 1920 = 20 columns of 96
NSQ = 6                     # phase-1 chunks whose s^2 runs on ACT (rest on DVE)

_cache = {}


def _runner(nc, n_cores):
    """Build a cached jitted callable for this Bass program via PJRT
    (mirrors concourse.bass2jax.run_bass_via_pjrt, but reusable)."""
    import jax
    from concourse import mybir
    from concourse.bass2jax import (
        _bass_exec_p,
        install_neuronx_cc_hook,
        partition_id_tensor,
    )
    from jax.sharding import Mesh, PartitionSpec
    from jax.experimental.shard_map import shard_map

    install_neuronx_cc_hook()
    partition_name = nc.partition_id_tensor.name if nc.partition_id_tensor else None

    in_names, out_names, out_avals = [], [], []
    for alloc in nc.m.functions[0].allocations:
        if not isinstance(alloc, mybir.MemoryLocationSet):
            continue
        name = alloc.memorylocations[0].name
        if alloc.kind == "ExternalInput":
            if name != partition_name:
                in_names.append(name)
        elif alloc.kind == "ExternalOutput":
            out_names.append(name)
            out_avals.append(
                jax.core.ShapedArray(
                    tuple(alloc.tensor_shape), mybir.dt.np(alloc.dtype)
                )
            )
    n_params = len(in_names)
    n_outs = len(out_avals)
    all_names = in_names + out_names + ([partition_name] if partition_name else [])
    donate = tuple(range(n_params, n_params + n_outs))

    def _body(*args):
        operands = list(args)
        if partition_name is not None:
            operands.append(partition_id_tensor())
        outs = _bass_exec_p.bind(
            *operands,
            out_avals=tuple(out_avals),
            in_names=tuple(all_names),
            out_names=tuple(out_names),
            lowering_input_output_aliases=(),
            sim_require_finite=True,
            sim_require_nnan=True,
            nc=nc,
        )
        return tuple(outs)

    devices = jax.devices()[:n_cores]
    mesh = Mesh(np.asarray(devices), ("core",))
    in_specs = (PartitionSpec("core"),) * (n_params + n_outs)
    out_specs = (PartitionSpec("core"),) * n_outs
    sharded = jax.jit(
        shard_map(
            _body, mesh=mesh, in_specs=in_specs, out_specs=out_specs, check_rep=False
        ),
        donate_argnums=donate,
        keep_unused=True,
    )

    def run(in_maps):
        concat_in = [
            np.concatenate([np.asarray(m[name]) for m in in_maps], axis=0)
            for name in in_names
        ]
        zeros = [
            np.zeros((n_cores * a.shape[0], *a.shape[1:]), a.dtype) for a in out_avals
        ]
        out_arrs = sharded(*concat_in, *zeros)
        return [
            {
                name: np.asarray(out_arrs[i]).reshape(
                    n_cores, *out_avals[i].shape
                )[c]
                for i, name in enumerate(out_names)
            }
            for c in range(n_cores)
        ]

    return run


def build_phase1():
    from contextlib import ExitStack

    import concourse.tile as tile
    from concourse import bacc, mybir

    f32 = mybir.dt.float32
    nc = bacc.Bacc("TRN2", target_bir_lowering=False, debug=False,
                   num_devices=N_CORES)
    x = nc.dram_tensor("x", (C, HW), f32, kind="ExternalInput").ap()
    rows = nc.dram_tensor("rows", (C, H), f32, kind="ExternalOutput").ap()
    ssq = nc.dram_tensor("ssq", (C, NCH1), f32, kind="ExternalOutput").ap()
    HC = H // NCH1  # rows per chunk

    with tile.TileContext(nc) as tc, ExitStack() as ctx:
        px = ctx.enter_context(tc.tile_pool(name="px", bufs=3))
        ps = ctx.enter_context(tc.tile_pool(name="ps", bufs=3))
        psq = ctx.enter_context(tc.tile_pool(name="psq", bufs=2))
        psm = ctx.enter_context(tc.tile_pool(name="psm", bufs=1))

        rows_sb = psm.tile([C, H], f32)
        ssq_sb = psm.tile([C, NCH1], f32)
        for i in range(NCH1):
            xt = px.tile([C, CH1], f32, tag="x")
            nc.sync.dma_start(xt[:], x[:, i * CH1:(i + 1) * CH1])
            st = ps.tile([C, CH1], f32, tag="s")
            nc.scalar.activation(st[:], xt[:], mybir.ActivationFunctionType.Sigmoid)
            nc.vector.tensor_reduce(
                rows_sb[:, i * HC:(i + 1) * HC],
                st[:].rearrange("p (h w) -> p h w", w=W),
                mybir.AxisListType.X,
                mybir.AluOpType.add,
            )
            sq = psq.tile([C, CH1], f32, tag="sq")
            if i < NSQ:
                nc.scalar.activation(
                    sq[:], st[:], mybir.ActivationFunctionType.Square,
                    accum_out=ssq_sb[:, i:i + 1],
                )
            else:
                nc.vector.scalar_tensor_tensor(
                    sq[:], st[:], 1.0, st[:],
                    op0=mybir.AluOpType.mult, op1=mybir.AluOpType.mult,
                    accum_out=ssq_sb[:, i:i + 1],
                )
        nc.sync.dma_start(rows[:], rows_sb[:])
        nc.sync.dma_start(ssq[:], ssq_sb[:])
    nc.compile()
    return nc


def build_phase2():
    from contextlib import ExitStack

    import concourse.tile as tile
    from concourse import bacc, mybir

    f32 = mybir.dt.float32
    u8 = mybir.dt.uint8
    Alu = mybir.AluOpType
    Act = mybir.ActivationFunctionType
    nc = bacc.Bacc("TRN2", target_bir_lowering=False, debug=False,
                   num_devices=N_CORES)
    sel = nc.dram_tensor("sel", (C, S2), f32, kind="ExternalInput").ap()
    msk = nc.dram_tensor("msk", (C, S2), u8, kind="ExternalInput").ap()
    ref = nc.dram_tensor("ref", (C, S2), f32, kind="ExternalOutput").ap()
    NB = CH2 // 32  # 32-col blocks per chunk (60)

    with tile.TileContext(nc) as tc, ExitStack() as ctx:
        pools = {}
        for name, bufs in [("sel", NCH2 + 1), ("m", 2), ("qa", 2),
                           ("vw", NCH2), ("d", NCH2 + 1), ("tb", 3),
                           ("wb", 2), ("rf", 2), ("sm", 1)]:
            pools[name] = ctx.enter_context(tc.tile_pool(name=name, bufs=bufs))

        psm = pools["sm"]
        mxp = psm.tile([C, NCH2], f32)
        mxr = psm.tile([C, 1], f32)
        mrow = psm.tile([1, C], f32)
        Mc = psm.tile([1, NS], f32)
        zc = psm.tile([1, NS], f32)
        den = psm.tile([1, NS], f32)
        rc1 = psm.tile([1, NS], f32)
        rc4 = psm.tile([1, C], f32)
        rcp = psm.tile([C, 1], f32)
        wred = psm.tile([C, NCH2 * NB], f32)

        selts, mts, dts, ats = [], [], [], []
        # --- stage I: load, q, scan, d, |d|, per-chunk max ---
        for i in range(NCH2):
            sl = slice(i * CH2, (i + 1) * CH2)
            selt = pools["sel"].tile([C, CH2], f32, tag="sel")
            nc.sync.dma_start(selt[:], sel[:, sl])
            mt = pools["m"].tile([C, CH2], u8, tag="m")
            nc.sync.dma_start(mt[:], msk[:, sl])
            qt = pools["qa"].tile([C, CH2], f32, tag="qa")
            # q = (m == 0) * sel
            nc.vector.scalar_tensor_tensor(
                qt[:], mt[:], 0.0, selt[:], op0=Alu.is_equal, op1=Alu.mult)
            Vt = pools["vw"].tile([C, CH2], f32, tag="vw")
            # state = m*state + q   (bottom-up propagation, per column)
            nc.vector.tensor_tensor_scan(
                Vt[:], mt[:], qt[:], 0.0, op0=Alu.mult, op1=Alu.add)
            dt = pools["d"].tile([C, CH2], f32, tag="d")
            nc.gpsimd.tensor_tensor(dt[:], Vt[:], selt[:], Alu.subtract)
            nc.vector.tensor_reduce(
                mxp[:, i:i + 1], dt[:], mybir.AxisListType.X, Alu.max,
                apply_absolute_value=True)
            selts.append(selt); mts.append(mt); dts.append(dt)

        # --- barrier: per-(b,c) max over space -> 1/m_clip per channel ---
        nc.vector.tensor_reduce(mxr[:], mxp[:], mybir.AxisListType.X, Alu.max)
        nc.sync.dma_start(mrow[:], mxr[:])  # (128,1) -> (1,128)
        nc.vector.tensor_reduce(
            Mc[:], mrow[:].rearrange("o (q c) -> o c q", q=WQ),
            mybir.AxisListType.X, Alu.max)
        nc.vector.tensor_scalar(zc[:], Mc[:], 0.0, None, op0=Alu.is_equal)
        nc.vector.scalar_tensor_tensor(
            den[:], Mc[:], CLIP, zc[:], op0=Alu.mult, op1=Alu.add)
        nc.vector.reciprocal(rc1[:], den[:])
        # broadcast (1,32) -> (1,128) on DVE, then DMA to per-partition (128,1)
        nc.vector.tensor_copy(
            rc4[:].rearrange("o (q c) -> o q c", q=WQ),
            rc1[:].unsqueeze(1).broadcast_to((1, WQ, NS)))
        nc.sync.dma_start(rcp[:], rc4[:])

        # --- stage II: w_px, channel max via 32x32 transpose, blend ---
        for i in range(NCH2):
            sl = slice(i * CH2, (i + 1) * CH2)
            # |d| / m_clip   (clip to 1 happens after the channel max)
            ad = pools["qa"].tile([C, CH2], f32, tag="qa")
            nc.scalar.activation(ad[:], dts[i][:], Act.Abs)
            wpx = pools["vw"].tile([C, CH2], f32, tag="vw")
            nc.vector.tensor_scalar(
                wpx[:], ad[:], rcp[:], None, op0=Alu.mult)
            t1 = pools["tb"].tile([C, CH2], f32, tag="tb")
            nc.vector.transpose(t1[:], wpx[:])
            wsl = slice(i * NB, (i + 1) * NB)
            nc.vector.tensor_reduce(
                wred[:, wsl], t1[:].rearrange("p (b c) -> p b c", c=32),
                mybir.AxisListType.X, Alu.max)
            # w = min(max_c w_px, 1)
            nc.vector.tensor_scalar(
                wred[:, wsl], wred[:, wsl], 1.0, None, op0=Alu.min)
            wexp = pools["wb"].tile([C, CH2], f32, tag="wbx")
            nc.vector.tensor_copy(
                wexp[:].rearrange("p (b c) -> p b c", c=32),
                wred[:, wsl].unsqueeze(-1).broadcast_to((C, NB, 32)))
            wb = pools["wb"].tile([C, CH2], f32, tag="wb")
            nc.vector.transpose(wb[:], wexp[:])
            tt = pools["tb"].tile([C, CH2], f32, tag="tb")
            nc.vector.tensor_tensor(tt[:], wb[:], dts[i][:], Alu.mult)
            rf = pools["rf"].tile([C, CH2], f32, tag="rf")
            nc.gpsimd.tensor_tensor(rf[:], tt[:], selts[i][:], Alu.add)
            nc.sync.dma_start(ref[:, sl], rf[:])
    nc.compile()
    return nc


# disparity ramp: jnp.linspace(0.1, 1.0, 96, dtype=float32) values
def _disp_f32():
    return np.linspace(0.1, 1.0, H).astype(np.float32)


def _select_channels(rows_sum_f64, ssq_f64):
    """Host-side ranking. rows_sum_f64: (C, H) summed over cores/batches,
    ssq_f64: (C,)."""
    disp = _disp_f32().astype(np.float64)
    depth = 1.0 - disp
    n_rep = B * W  # each h value appears B*W times in the full flattened vec
    dot_disp = rows_sum_f64 @ disp
    dot_depth = rows_sum_f64 @ depth
    vn_disp = np.sqrt(n_rep * (disp @ disp))
    vn_depth = np.sqrt(n_rep * (depth @ depth))
    sn = np.maximum(np.sqrt(ssq_f64), EPS)
    cos_disp = dot_disp / (sn * vn_disp)
    cos_depth = dot_depth / (sn * vn_depth)
    disp_idx = np.argsort(-cos_disp, kind="stable")[:NSEL]
    depth_idx = np.argsort(-cos_depth, kind="stable")[:NSEL]
    return np.concatenate([disp_idx, depth_idx])


def _pack_phase2_inputs(input_features, dynamic_masks, idx):
    """Pack selected channels and mask into the per-core (128, 7680) device
    layout: partition p = wq*32 + ch, free t = w'*96 + (95 - h)."""
    sel = input_features[:, idx]                       # (B, 32, H, W)
    sel_t = sel[:, :, ::-1, :].transpose(0, 1, 3, 2)   # (B, 32, W, Hrev)
    sel_p = np.ascontiguousarray(
        sel_t.reshape(B, NS, WQ, WPQ, H).transpose(0, 2, 1, 3, 4)
    ).reshape(B, C, S2)

    m_r = (dynamic_masks[:, ::-1, :] != 0).astype(np.uint8)  # (B, Hrev, W)
    m_r = m_r.copy()
    m_r[:, 0, :] = 0                # force reset at each column's bottom row
    m_t = m_r.transpose(0, 2, 1)    # (B, W, Hrev)
    m_q = np.ascontiguousarray(m_t).reshape(B, WQ, S2)
    m_big = np.broadcast_to(m_q[:, :, None, :], (B, WQ, NS, S2))
    m_big = np.ascontiguousarray(m_big).reshape(B, C, S2)
    return sel_p, m_big


def _unpack_refined(ref_stack):
    """(B, 128, 7680) device layout -> (B, 32, H, W)."""
    r = ref_stack.reshape(B, WQ, NS, WPQ, H).transpose(0, 2, 1, 3, 4)
    r = r.reshape(B, NS, W, H).transpose(0, 1, 3, 2)   # (B, 32, Hrev, W)
    return r[:, :, ::-1, :]


def _get_runners():
    if "run1" not in _cache:
        nc1 = build_phase1()
        _cache["run1"] = _runner(nc1, N_CORES)
        nc2 = build_phase2()
        _cache["run2"] = _runner(nc2, N_CORES)
    return _cache["run1"], _cache["run2"]


def _max_masked_run(dynamic_masks):
    """Longest run of consecutive masked rows in any column."""
    m = (dynamic_masks != 0)
    best = np.zeros((B, W), dtype=np.int32)
    cur = np.zeros((B, W), dtype=np.int32)
    for h in range(H - 1, -1, -1):
        cur = np.where(m[:, h, :], cur + 1, 0)
        best = np.maximum(best, cur)
    return int(best.max())


def kernel(input_features, dynamic_masks):
    input_features = np.asarray(input_features, dtype=np.float32)
    dynamic_masks = np.asarray(dynamic_masks)
    run1, run2 = _get_runners()

    # Phase 1: per-channel reductions on device
    in_maps1 = [
        {"x": input_features[b].reshape(C, HW)} for b in range(B)
    ]
    outs1 = run1(in_maps1)
    rows_sum = np.zeros((C, H), dtype=np.float64)
    ssq = np.zeros((C,), dtype=np.float64)
    for o in outs1:
        rows_sum += o["rows"].astype(np.float64)
        ssq += o["ssq"].astype(np.float64).sum(axis=1)
    idx = _select_channels(rows_sum, ssq)

    # the single-scan propagation is exact iff no masked run >= 33
    assert _max_masked_run(dynamic_masks) <= 32, (
        "masked run of >= 33 rows: single-scan shortcut invalid for this input"
    )

    # Phase 2: propagation + blend on device
    sel_p, m_big = _pack_phase2_inputs(input_features, dynamic_masks, idx)
    in_maps2 = [{"sel": sel_p[b], "msk": m_big[b]} for b in range(B)]
    outs2 = run2(in_maps2)
    ref_stack = np.stack([o["ref"] for o in outs2])
    refined = _unpack_refined(ref_stack)

    out = input_features.copy()
    out[:, idx] = refined
    return out


# revision 16
# speedup vs baseline: 18672.0201x; 18672.0201x over previous
"""Trainium2 Bass kernel for nn_GroundPropagation.

Structure (8 NeuronCores, batch-parallel, one batch element per core):

Phase 1 (device): per-channel reductions of s = sigmoid(x):
  - row sums  (C, H)  : sum over W of s           -> host computes disp/depth dots
  - sum of s^2 (C,)   : for the channel norms
Host: combines per-core partials in f64, ranks channels by cosine
  similarity against the disparity/depth ramps, picks top-16 + top-16.

Phase 2 (device): for the 32 selected channels, the 32-iteration masked
  "pull value from the row below" propagation collapses algebraically to
  a single bottom-up first-order recurrence per column:
      state = m[t] * state + (1 - m[t]) * sel[t]
  which is exactly one `tensor_tensor_scan` instruction per core
  (columns are packed per-partition; the mask is forced to 0 at each
  column's bottom row so the state resets at column boundaries).
  This is exact whenever no column has >= 33 consecutive masked rows
  (checked on host; P ~ 2^-33 per site otherwise).
  Then the clip-normalized blend weight and the final mix are computed
  and written back; host scatters the 32 channels into the full tensor.
"""

import sys

sys.path.insert(0, "/opt/trn_rl_repo")

import numpy as np

B, C, H, W = 8, 128, 96, 320
HW = H * W                  # 30720
NSEL = 16
NS = 2 * NSEL               # 32 selected channels
CLIP = 0.3
EPS = 1e-6
N_CORES = 8

NCH1, CH1 = 8, HW // 8      # phase-1 chunks (128, 3840)
WQ = 4                      # w-quarters; partition p = wq*32 + ch
WPQ = W // WQ               # 80 columns per quarter
S2 = WPQ * H                # 7680 free elems per partition in phase 2
NCH2 = 4
CH2 = S2 // NCH2            # 1920 = 20 columns of 96
NSQ = 5                     # phase-1 chunks whose s^2 runs on ACT (rest on DVE)

_cache = {}


def _runner(nc, n_cores):
    """Build a cached jitted callable for this Bass program via PJRT
    (mirrors concourse.bass2jax.run_bass_via_pjrt, but reusable)."""
    import jax
    from concourse import mybir
    from concourse.bass2jax import (
        _bass_exec_p,
        install_neuronx_cc_hook,
        partition_id_tensor,
    )
    from jax.sharding import Mesh, PartitionSpec
    from jax.experimental.shard_map import shard_map

    install_neuronx_cc_hook()
    partition_name = nc.partition_id_tensor.name if nc.partition_id_tensor else None

    in_names, out_names, out_avals = [], [], []
    for alloc in nc.m.functions[0].allocations:
        if not isinstance(alloc, mybir.MemoryLocationSet):
            continue
        name = alloc.memorylocations[0].name
        if alloc.kind == "ExternalInput":
            if name != partition_name:
                in_names.append(name)
        elif alloc.kind == "ExternalOutput":
            out_names.append(name)
            out_avals.append(
                jax.core.ShapedArray(
                    tuple(alloc.tensor_shape), mybir.dt.np(alloc.dtype)
                )
            )
    n_params = len(in_names)
    n_outs = len(out_avals)
    all_names = in_names + out_names + ([partition_name] if partition_name else [])
    donate = tuple(range(n_params, n_params + n_outs))

    def _body(*args):
        operands = list(args)
        if partition_name is not None:
            operands.append(partition_id_tensor())
        outs = _bass_exec_p.bind(
            *operands,
            out_avals=tuple(out_avals),
            in_names=tuple(all_names),
            out_names=tuple(out_names),
            lowering_input_output_aliases=(),
            sim_require_finite=True,
            sim_require_nnan=True,
            nc=nc,
        )
        return tuple(outs)

    devices = jax.devices()[:n_cores]
    mesh = Mesh(np.asarray(devices), ("core",))
    in_specs = (PartitionSpec("core"),) * (n_params + n_outs)
    out_specs = (PartitionSpec("core"),) * n_outs
    sharded = jax.jit(
        shard_map(
            _body, mesh=mesh, in_specs=in_specs, out_specs=out_specs, check_rep=False
        ),
        donate_argnums=donate,
        keep_unused=True,
    )

    def run(in_maps):
        concat_in = [
            np.concatenate([np.asarray(m[name]) for m in in_maps], axis=0)
            for name in in_names
        ]
        zeros = [
            np.zeros((n_cores * a.shape[0], *a.shape[1:]), a.dtype) for a in out_avals
        ]
        out_arrs = sharded(*concat_in, *zeros)
        return [
            {
                name: np.asarray(out_arrs[i]).reshape(
                    n_cores, *out_avals[i].shape
                )[c]
                for i, name in enumerate(out_names)
            }
            for c in range(n_cores)
        ]

    return run


def build_phase1():
    from contextlib import ExitStack

    import concourse.tile as tile
    from concourse import bacc, mybir

    f32 = mybir.dt.float32
    nc = bacc.Bacc("TRN2", target_bir_lowering=False, debug=False,
                   num_devices=N_CORES)
    x = nc.dram_tensor("x", (C, HW), f32, kind="ExternalInput").ap()
    rows = nc.dram_tensor("rows", (C, H), f32, kind="ExternalOutput").ap()
    ssq = nc.dram_tensor("ssq", (C, NCH1), f32, kind="ExternalOutput").ap()
    HC = H // NCH1  # rows per chunk

    with tile.TileContext(nc) as tc, ExitStack() as ctx:
        px = ctx.enter_context(tc.tile_pool(name="px", bufs=3))
        ps = ctx.enter_context(tc.tile_pool(name="ps", bufs=3))
        psq = ctx.enter_context(tc.tile_pool(name="psq", bufs=2))
        psm = ctx.enter_context(tc.tile_pool(name="psm", bufs=1))

        rows_sb = psm.tile([C, H], f32)
        ssq_a = psm.tile([C, NSQ], f32)
        ssq_d = psm.tile([C, NCH1 - NSQ], f32)
        for i in range(NCH1):
            xt = px.tile([C, CH1], f32, tag="x")
            nc.sync.dma_start(xt[:], x[:, i * CH1:(i + 1) * CH1])
            st = ps.tile([C, CH1], f32, tag="s")
            nc.scalar.activation(st[:], xt[:], mybir.ActivationFunctionType.Sigmoid)
            nc.vector.tensor_reduce(
                rows_sb[:, i * HC:(i + 1) * HC],
                st[:].rearrange("p (h w) -> p h w", w=W),
                mybir.AxisListType.X,
                mybir.AluOpType.add,
            )
            sq = psq.tile([C, CH1], f32, tag="sq")
            if i < NSQ:
                nc.scalar.activation(
                    sq[:], st[:], mybir.ActivationFunctionType.Square,
                    accum_out=ssq_a[:, i:i + 1],
                )
            else:
                nc.vector.scalar_tensor_tensor(
                    sq[:], st[:], 1.0, st[:],
                    op0=mybir.AluOpType.mult, op1=mybir.AluOpType.mult,
                    accum_out=ssq_d[:, i - NSQ:i - NSQ + 1],
                )
        nc.sync.dma_start(rows[:], rows_sb[:])
        nc.sync.dma_start(ssq[:, :NSQ], ssq_a[:])
        nc.sync.dma_start(ssq[:, NSQ:], ssq_d[:])
    nc.compile()
    return nc


def build_phase2():
    from contextlib import ExitStack

    import concourse.tile as tile
    from concourse import bacc, mybir

    f32 = mybir.dt.float32
    u8 = mybir.dt.uint8
    Alu = mybir.AluOpType
    Act = mybir.ActivationFunctionType
    nc = bacc.Bacc("TRN2", target_bir_lowering=False, debug=False,
                   num_devices=N_CORES)
    sel = nc.dram_tensor("sel", (C, S2), f32, kind="ExternalInput").ap()
    msk = nc.dram_tensor("msk", (C, S2), u8, kind="ExternalInput").ap()
    ref = nc.dram_tensor("ref", (C, S2), f32, kind="ExternalOutput").ap()
    NB = CH2 // 32  # 32-col blocks per chunk (60)

    with tile.TileContext(nc) as tc, ExitStack() as ctx:
        pools = {}
        for name, bufs in [("sel", NCH2 + 1), ("m", 2), ("qa", 2),
                           ("vw", 3), ("d", NCH2 + 1), ("tb", 3),
                           ("wb", 3), ("wr", 3), ("rf", 3), ("sm", 1)]:
            pools[name] = ctx.enter_context(tc.tile_pool(name=name, bufs=bufs))
        for name in ("ps1", "ps2"):
            pools[name] = ctx.enter_context(
                tc.tile_pool(name=name, bufs=2, space="PSUM"))
        from concourse.masks import make_identity
        ident = pools["sm"].tile([C, C], f32)
        make_identity(nc, ident[:])

        psm = pools["sm"]
        mxp = psm.tile([C, NCH2], f32)
        mxr = psm.tile([C, 1], f32)
        mrow = psm.tile([1, C], f32)
        Mc = psm.tile([1, NS], f32)
        zc = psm.tile([1, NS], f32)
        den = psm.tile([1, NS], f32)
        rc1 = psm.tile([1, NS], f32)
        rc4 = psm.tile([1, C], f32)
        rcp = psm.tile([C, 1], f32)
        wred = psm.tile([C, NCH2 * NB], f32)

        selts, mts, dts, ats = [], [], [], []
        # --- stage I: load, q, scan, d, |d|, per-chunk max ---
        for i in range(NCH2):
            sl = slice(i * CH2, (i + 1) * CH2)
            selt = pools["sel"].tile([C, CH2], f32, tag="sel")
            nc.sync.dma_start(selt[:], sel[:, sl])
            mt = pools["m"].tile([C, CH2], u8, tag="m")
            nc.sync.dma_start(mt[:], msk[:, sl])
            qt = pools["qa"].tile([C, CH2], f32, tag="qa")
            # q = (m == 0) * sel
            nc.vector.scalar_tensor_tensor(
                qt[:], mt[:], 0.0, selt[:], op0=Alu.is_equal, op1=Alu.mult)
            Vt = pools["vw"].tile([C, CH2], f32, tag="vw")
            # state = m*state + q   (bottom-up propagation, per column)
            nc.vector.tensor_tensor_scan(
                Vt[:], mt[:], qt[:], 0.0, op0=Alu.mult, op1=Alu.add)
            dt = pools["d"].tile([C, CH2], f32, tag="d")
            nc.gpsimd.tensor_tensor(dt[:], Vt[:], selt[:], Alu.subtract)
            nc.vector.tensor_reduce(
                mxp[:, i:i + 1], dt[:], mybir.AxisListType.X, Alu.max,
                apply_absolute_value=True)
            selts.append(selt); mts.append(mt); dts.append(dt)

        # --- barrier: per-(b,c) max over space -> 1/m_clip per channel ---
        nc.vector.tensor_reduce(mxr[:], mxp[:], mybir.AxisListType.X, Alu.max)
        nc.sync.dma_start(mrow[:], mxr[:])  # (128,1) -> (1,128)
        nc.vector.tensor_reduce(
            Mc[:], mrow[:].rearrange("o (q c) -> o c q", q=WQ),
            mybir.AxisListType.X, Alu.max)
        nc.vector.tensor_scalar(zc[:], Mc[:], 0.0, None, op0=Alu.is_equal)
        nc.vector.scalar_tensor_tensor(
            den[:], Mc[:], CLIP, zc[:], op0=Alu.mult, op1=Alu.add)
        nc.vector.reciprocal(rc1[:], den[:])
        # broadcast (1,32) -> (1,128) on DVE, then DMA to per-partition (128,1)
        nc.vector.tensor_copy(
            rc4[:].rearrange("o (q c) -> o q c", q=WQ),
            rc1[:].unsqueeze(1).broadcast_to((1, WQ, NS)))
        nc.sync.dma_start(rcp[:], rc4[:])

        # --- stage II: w_px on ACT, channel max via PE transposes, blend ---
        SPLITS = [(0, 1024), (1024, 896)]  # 128-aligned sub-chunks per chunk
        for i in range(NCH2):
            for off, ln in SPLITS:
                nt = ln // 128
                sl = slice(i * CH2 + off, i * CH2 + off + ln)
                dsl = slice(off, off + ln)
                # w_px = |d| / m_clip on ACT (clip to 1 after the channel max)
                wpx = pools["vw"].tile([C, ln], f32, tag="vw",
                                       padded_shape=[C, 1024])
                nc.scalar.activation(wpx[:], dts[i][:, dsl], Act.Abs,
                                     scale=rcp[:])
                # transpose to (pos, (wq, ch)) on PE
                t1p = pools["ps1"].tile([C, ln], f32, tag="ps1", space="PSUM",
                                        padded_shape=[C, 1024])
                for t in range(nt):
                    ts = slice(t * 128, (t + 1) * 128)
                    nc.tensor.transpose(t1p[:, ts], wpx[:, ts], ident[:])
                # max over ch within each (tile, wq); then clip at 1
                wrd = pools["wr"].tile([C, nt * WQ], f32, tag="wr",
                                       padded_shape=[C, 32])
                nc.vector.tensor_reduce(
                    wrd[:], t1p[:].rearrange("p (t q c) -> p t q c",
                                             q=WQ, c=NS),
                    mybir.AxisListType.X, Alu.max)
                nc.vector.tensor_scalar(wrd[:], wrd[:], 1.0, None,
                                        op0=Alu.min)
                # broadcast back over ch and transpose back on PE
                wexp = pools["wb"].tile([C, ln], f32, tag="wbx",
                                        padded_shape=[C, 1024])
                nc.scalar.activation(
                    wexp[:].rearrange("p (t q c) -> p t q c", q=WQ, c=NS),
                    wrd[:].rearrange("p (t q) -> p t q", q=WQ).unsqueeze(-1)
                    .broadcast_to((C, nt, WQ, NS)),
                    Act.Copy)
                wbp = pools["ps2"].tile([C, ln], f32, tag="ps2", space="PSUM",
                                        padded_shape=[C, 1024])
                for t in range(nt):
                    ts = slice(t * 128, (t + 1) * 128)
                    nc.tensor.transpose(wbp[:, ts], wexp[:, ts], ident[:])
                tt = pools["tb"].tile([C, ln], f32, tag="tb",
                                      padded_shape=[C, 1024])
                nc.vector.tensor_tensor(tt[:], wbp[:], dts[i][:, dsl],
                                        Alu.mult)
                rf = pools["rf"].tile([C, ln], f32, tag="rf",
                                      padded_shape=[C, 1024])
                nc.gpsimd.tensor_tensor(rf[:], tt[:], selts[i][:, dsl],
                                        Alu.add)
                nc.sync.dma_start(ref[:, sl], rf[:])
    nc.compile()
    return nc


# disparity ramp: jnp.linspace(0.1, 1.0, 96, dtype=float32) values
def _disp_f32():
    return np.linspace(0.1, 1.0, H).astype(np.float32)


def _select_channels(rows_sum_f64, ssq_f64):
    """Host-side ranking. rows_sum_f64: (C, H) summed over cores/batches,
    ssq_f64: (C,)."""
    disp = _disp_f32().astype(np.float64)
    depth = 1.0 - disp
    n_rep = B * W  # each h value appears B*W times in the full flattened vec
    dot_disp = rows_sum_f64 @ disp
    dot_depth = rows_sum_f64 @ depth
    vn_disp = np.sqrt(n_rep * (disp @ disp))
    vn_depth = np.sqrt(n_rep * (depth @ depth))
    sn = np.maximum(np.sqrt(ssq_f64), EPS)
    cos_disp = dot_disp / (sn * vn_disp)
    cos_depth = dot_depth / (sn * vn_depth)
    disp_idx = np.argsort(-cos_disp, kind="stable")[:NSEL]
    depth_idx = np.argsort(-cos_depth, kind="stable")[:NSEL]
    return np.concatenate([disp_idx, depth_idx])


def _pack_phase2_inputs(input_features, dynamic_masks, idx):
    """Pack selected channels and mask into the per-core (128, 7680) device
    layout: partition p = wq*32 + ch, free t = w'*96 + (95 - h)."""
    sel = input_features[:, idx]                       # (B, 32, H, W)
    sel_t = sel[:, :, ::-1, :].transpose(0, 1, 3, 2)   # (B, 32, W, Hrev)
    sel_p = np.ascontiguousarray(
        sel_t.reshape(B, NS, WQ, WPQ, H).transpose(0, 2, 1, 3, 4)
    ).reshape(B, C, S2)

    m_r = (dynamic_masks[:, ::-1, :] != 0).astype(np.uint8)  # (B, Hrev, W)
    m_r = m_r.copy()
    m_r[:, 0, :] = 0                # force reset at each column's bottom row
    m_t = m_r.transpose(0, 2, 1)    # (B, W, Hrev)
    m_q = np.ascontiguousarray(m_t).reshape(B, WQ, S2)
    m_big = np.broadcast_to(m_q[:, :, None, :], (B, WQ, NS, S2))
    m_big = np.ascontiguousarray(m_big).reshape(B, C, S2)
    return sel_p, m_big


def _unpack_refined(ref_stack):
    """(B, 128, 7680) device layout -> (B, 32, H, W)."""
    r = ref_stack.reshape(B, WQ, NS, WPQ, H).transpose(0, 2, 1, 3, 4)
    r = r.reshape(B, NS, W, H).transpose(0, 1, 3, 2)   # (B, 32, Hrev, W)
    return r[:, :, ::-1, :]


def _get_runners():
    if "run1" not in _cache:
        nc1 = build_phase1()
        _cache["run1"] = _runner(nc1, N_CORES)
        nc2 = build_phase2()
        _cache["run2"] = _runner(nc2, N_CORES)
    return _cache["run1"], _cache["run2"]


def _max_masked_run(dynamic_masks):
    """Longest run of consecutive masked rows in any column."""
    m = (dynamic_masks != 0)
    best = np.zeros((B, W), dtype=np.int32)
    cur = np.zeros((B, W), dtype=np.int32)
    for h in range(H - 1, -1, -1):
        cur = np.where(m[:, h, :], cur + 1, 0)
        best = np.maximum(best, cur)
    return int(best.max())


def kernel(input_features, dynamic_masks):
    input_features = np.asarray(input_features, dtype=np.float32)
    dynamic_masks = np.asarray(dynamic_masks)
    run1, run2 = _get_runners()

    # Phase 1: per-channel reductions on device
    in_maps1 = [
        {"x": input_features[b].reshape(C, HW)} for b in range(B)
    ]
    outs1 = run1(in_maps1)
    rows_sum = np.zeros((C, H), dtype=np.float64)
    ssq = np.zeros((C,), dtype=np.float64)
    for o in outs1:
        rows_sum += o["rows"].astype(np.float64)
        ssq += o["ssq"].astype(np.float64).sum(axis=1)
    idx = _select_channels(rows_sum, ssq)

    # the single-scan propagation is exact iff no masked run >= 33
    assert _max_masked_run(dynamic_masks) <= 32, (
        "masked run of >= 33 rows: single-scan shortcut invalid for this input"
    )

    # Phase 2: propagation + blend on device
    sel_p, m_big = _pack_phase2_inputs(input_features, dynamic_masks, idx)
    in_maps2 = [{"sel": sel_p[b], "msk": m_big[b]} for b in range(B)]
    outs2 = run2(in_maps2)
    ref_stack = np.stack([o["ref"] for o in outs2])
    refined = _unpack_refined(ref_stack)

    out = input_features.copy()
    out[:, idx] = refined
    return out
